# revision 1
# baseline (speedup 1.0000x reference)
"""2-layer GAT (nn_GAT_88381837017178) on 8 trn2 NeuronCores via Bass/Tile.

Takes FULL unsharded inputs, returns the FULL [1,2] output.

Math: with x [N,1], h1 = x @ W1 is rank-1, so each head's attention
logits are affine in x and layer-1's aggregated output per head is
s1[n,h] * W1row[h,:], where s1 is the attention-weighted sum of x[src].
relu splits rank-2: relu(s1*W1row) = pos(s1)*posW1 + neg(s1)*negW1, so
layer 2 only needs a per-node 4-vector [h2_0, h2_1, as2, ad2], linear in
[pos(s1), neg(s1)] (valid because b1 == b2 == 0; asserted).

Sharding (edge parallelism per the dst-sorted contiguous split): nodes
are split over 8 cores; each core owns a dst-complete edge shard laid
out as dense node-padded [128, T, K] tiles (two degree buckets).  The
segment softmax becomes dense DVE/ACT tile ops.  Layer 2 needs node
features of arbitrary global src nodes: per-core [NPPC,4] table slices
are AllGather'd across the 8 cores, then an indirect (gather) DMA pulls
per-edge rows, and a second dense softmax + logsoftmax partial-sum
epilogue finishes on-device.  Host only sorts/pads indices, gathers the
raw input x per edge slot, and sums the 8 [128,4] per-core partials.

If anything in the device path fails (different shapes, nonzero biases,
wedged device), kernel() falls back to an equivalent numpy
implementation so correctness is preserved.
"""
import os
import numpy as np

N = 50000
E = 400000
NC = 8
NPC = N // NC
H1, F1 = 8, 64
P = 128

LAST_EXEC_NS = None


def _ceil(a, b):
    return -(-a // b)


# ---------------------------------------------------------------- host prep
def _host_prep(x, edge_index, W1, a_src1, a_dst1, W2, a_src2, a_dst2):
    import ml_dtypes

    x = np.asarray(x, np.float32).reshape(-1)
    ei = np.asarray(edge_index)
    W1 = np.asarray(W1, np.float32).reshape(H1, F1)
    a_src1 = np.asarray(a_src1, np.float32)
    a_dst1 = np.asarray(a_dst1, np.float32)
    W2 = np.asarray(W2, np.float32).reshape(H1, F1, 2)
    a_src2 = np.asarray(a_src2, np.float32).reshape(2)
    a_dst2 = np.asarray(a_dst2, np.float32).reshape(2)

    loop = np.arange(N, dtype=np.int64)
    src = np.concatenate([ei[0].astype(np.int64), loop])
    dst = np.concatenate([ei[1].astype(np.int64), loop])
    order = np.argsort(dst, kind="stable")
    src_s = src[order]
    deg = np.bincount(dst, minlength=N).astype(np.int64)
    starts = np.zeros(N + 1, np.int64)
    np.cumsum(deg, out=starts[1:])

    kmax = int(deg.max())
    K2 = _ceil(kmax, 4) * 4
    best = None
    for K1 in range(8, min(K2, 28) + 1, 4):
        degc = deg.reshape(NC, NPC)
        c1 = (degc <= K1).sum(axis=1)
        c2 = NPC - c1
        T1 = _ceil(int(c1.max()), P)
        T2 = _ceil(int(c2.max()), P) + 1     # +1 pad tile guarantees dummy row
        slots = T1 * P * K1 + T2 * P * K2
        if best is None or slots < best[0]:
            best = (slots, K1, T1, T2)
    _, K1, T1, T2 = best
    T = T1 + T2
    NPPC = T * P
    DUMMY = NPPC - 1

    cs = np.einsum("hf,hf->h", W1, a_src1).astype(np.float32)
    cd = np.einsum("hf,hf->h", W1, a_dst1).astype(np.float32)
    A = np.einsum("hf,hfc->hc", np.maximum(W1, 0.0), W2)
    Bm = np.einsum("hf,hfc->hc", np.maximum(-W1, 0.0), W2)
    wvec = np.zeros((4, 16), np.float32)
    wvec[0] = np.concatenate([A[:, 0], Bm[:, 0]])
    wvec[1] = np.concatenate([A[:, 1], Bm[:, 1]])
    wvec[2] = np.concatenate([A @ a_src2, Bm @ a_src2])
    wvec[3] = np.concatenate([A @ a_dst2, Bm @ a_dst2])

    def to_pt(v):
        return np.ascontiguousarray(v.reshape(T, P).T)

    def to_ptk(v, Tb, Kb):
        return np.ascontiguousarray(
            v.reshape(Tb, P, Kb).transpose(1, 0, 2).reshape(P, Tb * Kb))

    tabrow = np.zeros(N, np.int64)
    percore = []
    for c in range(NC):
        n0 = c * NPC
        degc = deg[n0:n0 + NPC]
        b1_ids = np.nonzero(degc <= K1)[0] + n0
        b2_ids = np.nonzero(degc > K1)[0] + n0
        rows = np.zeros(NPC, np.int64)
        rows[b1_ids - n0] = np.arange(len(b1_ids))
        rows[b2_ids - n0] = T1 * P + np.arange(len(b2_ids))
        tabrow[n0:n0 + NPC] = c * NPPC + rows
        percore.append((b1_ids, b2_ids, rows))

    cores = []
    for c in range(NC):
        n0 = c * NPC
        b1_ids, b2_ids, rows = percore[c]
        xd = np.zeros(NPPC, np.float32)
        nm = np.zeros(NPPC, np.float32)
        pc_ = np.zeros(NPPC, np.float32)
        xd[rows] = x[n0:n0 + NPC]
        nm[rows] = 1.0
        kb_of_row = np.where(np.arange(NPPC) < T1 * P, K1, K2).astype(np.float32)
        pc_[:] = kb_of_row - 1.0             # pad rows pretend degree 1
        pc_[rows] = kb_of_row[rows] - deg[n0:n0 + NPC]

        def edge_tables(ids, Tb, Kb, row_off):
            xs = np.zeros((Tb * P, Kb), np.float32)
            gi = np.full((Tb * P, Kb), DUMMY, np.int64)
            if len(ids):
                d = deg[ids]
                rep_rows = np.repeat(rows[ids - n0] - row_off, d)
                k = np.arange(d.sum()) - np.repeat(
                    np.concatenate([[0], d.cumsum()[:-1]]), d)
                epos = np.repeat(starts[ids], d) + k
                s_nodes = src_s[epos]
                xs[rep_rows, k] = x[s_nodes]
                gi[rep_rows, k] = tabrow[s_nodes]
            return to_ptk(xs, Tb, Kb), to_ptk(gi, Tb, Kb).astype(np.int32)

        xs1, gi1 = edge_tables(b1_ids, T1, K1, 0)
        xs2, gi2 = edge_tables(b2_ids, T2, K2, T1 * P)
        cores.append(dict(
            xs1=xs1.astype(ml_dtypes.bfloat16), gi1=gi1,
            xs2=xs2.astype(ml_dtypes.bfloat16), gi2=gi2,
            xdT=to_pt(xd), nmT=to_pt(nm), pcT=to_pt(pc_),
            csrep=np.tile(cs, (P, 1)), cdrep=np.tile(cd, (P, 1)),
            bsdrep=np.zeros((P, 8), np.float32),
            wvec=np.tile(wvec.reshape(1, 64), (P, 1)),
            dumrow=np.array([[0.0, 0.0, -1e30, 0.0]], ml_dtypes.bfloat16),
        ))

    meta = dict(K1=K1, K2=K2, T1=T1, T2=T2, T=T, NPPC=NPPC, DUMMY=DUMMY)
    return meta, cores


# ---------------------------------------------------------------- program
def _build_program(meta, gather_chunks=2):
    import concourse.bacc as bacc
    import concourse.tile as tile
    from concourse import mybir
    from concourse.bass import IndirectOffsetOnAxis

    F32 = mybir.dt.float32
    BF16 = mybir.dt.bfloat16
    I32 = mybir.dt.int32
    AX = mybir.AxisListType
    OP = mybir.AluOpType
    AF = mybir.ActivationFunctionType

    K1, K2, T1, T2, T = meta["K1"], meta["K2"], meta["T1"], meta["T2"], meta["T"]
    NPPC = meta["NPPC"]
    NTAB = NC * NPPC

    nc = bacc.Bacc("TRN2", target_bir_lowering=False, debug=False,
                   num_devices=NC)
    ins = {}
    for name, shape, dt in [
        ("xs1", [P, T1 * K1], BF16), ("gi1", [P, T1 * K1], I32),
        ("xs2", [P, T2 * K2], BF16), ("gi2", [P, T2 * K2], I32),
        ("xdT", [P, T], F32), ("nmT", [P, T], F32), ("pcT", [P, T], F32),
        ("csrep", [P, 8], F32), ("cdrep", [P, 8], F32), ("bsdrep", [P, 8], F32),
        ("wvec", [P, 64], F32), ("dumrow", [1, 4], BF16),
    ]:
        ins[name] = nc.dram_tensor(name, shape, dt, kind="ExternalInput").ap()
    out_part = nc.dram_tensor("part", [P, 4], F32, kind="ExternalOutput").ap()

    _lp = nc.allow_low_precision("bf16 edge softmax; final tolerance 2e-2")
    _lp.__enter__()
    with tile.TileContext(nc) as tc:
        with tc.tile_pool(name="sb", bufs=1) as pool, \
             tc.tile_pool(name="dram", bufs=1, space="DRAM") as dram:
            sb = {}
            order = ["dumrow", "xdT", "csrep", "cdrep", "bsdrep",
                     "pcT", "nmT", "wvec", "xs1", "xs2", "gi1", "gi2"]
            wu = {}
            for name in order:
                ap = ins[name]
                t_ = pool.tile(list(ap.shape), ap.dtype, name=f"sb_{name}")
                nc.sync.dma_start(out=t_[:], in_=ap[:])
                sb[name] = t_
                if name == "dumrow":
                    # warm-up collective first: absorbs core launch skew and
                    # ncfw first-use cost while the big input DMAs stream in
                    wu["in"] = dram.tile([1, 4], BF16, name="wuin")
                    wu["out"] = dram.tile([NC, 4], BF16, name="wuout",
                                          addr_space="Shared")
                    nc.sync.dma_start(out=wu["in"][:], in_=t_[:])
                    nc.gpsimd.collective_compute(
                        "AllGather", OP.bypass,
                        replica_groups=[list(range(NC))],
                        ins=[wu["in"].opt()], outs=[wu["out"].opt()])

            buckets = [
                (sb["xs1"], sb["gi1"], T1, K1, 0),
                (sb["xs2"], sb["gi2"], T2, K2, T1),
            ]

            xdcd = pool.tile([P, 8 * T], F32)
            xdcd3 = xdcd[:].rearrange("p (h t) -> p h t", h=8)
            for h in range(8):
                nc.scalar.activation(xdcd3[:, h, :], sb["xdT"][:], AF.Identity,
                                     bias=sb["bsdrep"][:, h:h + 1],
                                     scale=sb["cdrep"][:, h:h + 1])

            num = pool.tile([P, T * 8], BF16)
            den = pool.tile([P, T * 8], BF16)
            num3 = num[:].rearrange("p (h t) -> p h t", h=8)
            den3 = den[:].rearrange("p (h t) -> p h t", h=8)

            scratch = {}
            for bi, (xs_sb, gi_sb, Tb, Kb, t0) in enumerate(buckets):
                scratch[bi] = dict(
                    u=pool.tile([P, Tb * Kb], BF16, name=f"u_{bi}"),
                    l=pool.tile([P, Tb * Kb], BF16, name=f"l_{bi}"),
                    ex=pool.tile([P, Tb * Kb], BF16, name=f"ex_{bi}"),
                    m=pool.tile([P, Tb * Kb], BF16, name=f"m_{bi}"),
                )

            # ---- layer-1 edge phase (dense segment softmax, all 8 heads)
            # per-head u tiles so all Lrelu ops batch under one ACT table
            ut = {}
            for h in range(8):
                for bi, (xs_sb, gi_sb, Tb, Kb, t0) in enumerate(buckets):
                    ut[h, bi] = pool.tile([P, Tb * Kb], BF16, name=f"u{h}_{bi}")
            for h in range(8):
                for bi, (xs_sb, gi_sb, Tb, Kb, t0) in enumerate(buckets):
                    xs3 = xs_sb[:].rearrange("p (t k) -> p t k", k=Kb)
                    u3 = ut[h, bi][:].rearrange("p (t k) -> p t k", k=Kb)
                    xdcd_b = xdcd3[:, h, t0:t0 + Tb].to_broadcast([P, Tb, Kb])
                    nc.vector.scalar_tensor_tensor(
                        out=u3, in0=xs3, scalar=sb["csrep"][:, h:h + 1],
                        in1=xdcd_b, op0=OP.mult, op1=OP.add)
            # all leaky-relus back-to-back (single ACT table load)
            for h in range(8):
                for bi in range(len(buckets)):
                    nc.scalar.activation(ut[h, bi][:], ut[h, bi][:],
                                         AF.Lrelu, alpha=0.2)
                nc.scalar.activation(xdcd3[:, h, :], xdcd3[:, h, :],
                                     AF.Lrelu, alpha=0.2)
            # exp + weighted sums (back on the exp table)
            for h in range(8):
                for bi, (xs_sb, gi_sb, Tb, Kb, t0) in enumerate(buckets):
                    ss = scratch[bi]
                    nc.scalar.activation(ss["ex"][:], ut[h, bi][:], AF.Exp)
                    nc.vector.tensor_tensor(out=ss["m"][:], in0=ss["ex"][:],
                                            in1=xs_sb[:], op=OP.mult)
                    m3 = ss["m"][:].rearrange("p (t k) -> p t k", k=Kb)
                    ex3 = ss["ex"][:].rearrange("p (t k) -> p t k", k=Kb)
                    nc.vector.tensor_reduce(out=num3[:, h, t0:t0 + Tb],
                                            in_=m3, axis=AX.X, op=OP.add)
                    nc.vector.tensor_reduce(out=den3[:, h, t0:t0 + Tb],
                                            in_=ex3, axis=AX.X, op=OP.add)
                # pad-slot correction (xdcd already leaky-relu'd in place)
                et = pool.tile([P, T], BF16, name="et", tag="et")
                nc.scalar.activation(et[:], xdcd3[:, h, :], AF.Exp)
                nc.vector.tensor_tensor(out=et[:], in0=et[:],
                                        in1=sb["pcT"][:], op=OP.mult)
                nc.vector.tensor_tensor(out=den3[:, h, :], in0=den3[:, h, :],
                                        in1=et[:], op=OP.subtract)

            # ---- s1 = num/den; pm = [relu(s1), relu(-s1)]
            rec = pool.tile([P, T * 8], F32)
            nc.vector.reciprocal(out=rec[:], in_=den[:])
            s1 = pool.tile([P, T * 8], F32)
            nc.vector.tensor_tensor(out=s1[:], in0=num[:], in1=rec[:], op=OP.mult)
            s13 = s1[:].rearrange("p (h t) -> p t h", h=8)
            pm = pool.tile([P, T * 16], F32)
            pm3 = pm[:].rearrange("p (t j) -> p t j", j=16)
            nc.scalar.activation(pm3[:, :, 0:8], s13, AF.Relu)
            nc.scalar.activation(pm3[:, :, 8:16], s13, AF.Relu, scale=-1.0)

            # ---- per-node table row [h2_0, h2_1, as2, ad2] = pm @ wvec.T
            cols = pool.tile([P, T * 4], BF16)
            cols3 = cols[:].rearrange("p (t c) -> p t c", c=4)
            wv4 = sb["wvec"][:].rearrange("p (c j) -> p c j", c=4)
            for cc in range(4):
                pr = pool.tile([P, T * 16], F32, name="pr", tag="pr")
                pr3 = pr[:].rearrange("p (t j) -> p t j", j=16)
                wb = wv4[:, cc, :].unsqueeze(1).to_broadcast([P, T, 16])
                nc.vector.tensor_tensor(out=pr3, in0=pm3, in1=wb, op=OP.mult)
                nc.vector.tensor_reduce(out=cols3[:, :, cc], in_=pr3,
                                        axis=AX.X, op=OP.add)

            # ---- local table slice -> AllGather -> full table
            tabloc = dram.tile([NPPC, 4], BF16)
            tabv = tabloc[:].rearrange("(t p) c -> p t c", p=P)
            nc.sync.dma_start(out=tabv, in_=cols3)
            # every core's last row is padding; it is the gather dummy row
            nc.sync.dma_start(out=tabloc[NPPC - 1:NPPC, :], in_=sb["dumrow"][:])
            tabfull = dram.tile([NTAB, 4], BF16, addr_space="Shared")
            nc.gpsimd.collective_compute(
                "AllGather", OP.bypass, replica_groups=[list(range(NC))],
                ins=[tabloc.opt()], outs=[tabfull.opt()])

            # ---- layer-2: chunked indirect gather + dense segment softmax
            n0t = pool.tile([P, T], F32)
            n1t = pool.tile([P, T], F32)
            d2t = pool.tile([P, T], F32)
            for bi, (xs_sb, gi_sb, Tb, Kb, t0) in enumerate(buckets):
                g4 = pool.tile([P, Tb * Kb * 4], BF16, name=f"g4_{bi}")
                g44 = g4[:].rearrange("p (t k c) -> p t k c", k=Kb, c=4)
                s = scratch[bi]
                u3a = s["u"][:].rearrange("p (t k) -> p t k", k=Kb)
                l3a = s["l"][:].rearrange("p (t k) -> p t k", k=Kb)
                ex3a = s["ex"][:].rearrange("p (t k) -> p t k", k=Kb)
                m3a = s["m"][:].rearrange("p (t k) -> p t k", k=Kb)
                step = _ceil(Tb, gather_chunks)
                for tch in range(0, Tb, step):
                    te = min(tch + step, Tb)
                    cs_ = slice(tch, te)
                    nc.gpsimd.indirect_dma_start(
                        out=g4[:, tch * Kb * 4:te * Kb * 4],
                        out_offset=None,
                        in_=tabfull[:],
                        in_offset=IndirectOffsetOnAxis(
                            ap=gi_sb[:, tch * Kb:te * Kb], axis=0))
                    ad2b = cols3[:, t0 + tch:t0 + te, 3].to_broadcast(
                        [P, te - tch, Kb])
                    nc.vector.tensor_tensor(out=u3a[:, cs_], in0=g44[:, cs_, :, 2],
                                            in1=ad2b, op=OP.add)
                    nc.scalar.activation(l3a[:, cs_], u3a[:, cs_], AF.Exp, scale=0.2)
                    nc.scalar.activation(ex3a[:, cs_], u3a[:, cs_], AF.Exp)
                    nc.vector.tensor_tensor(out=ex3a[:, cs_], in0=ex3a[:, cs_],
                                            in1=l3a[:, cs_], op=OP.max)
                    nc.vector.tensor_reduce(out=d2t[:, t0 + tch:t0 + te],
                                            in_=ex3a[:, cs_], axis=AX.X, op=OP.add)
                    nc.vector.tensor_tensor(out=m3a[:, cs_], in0=ex3a[:, cs_],
                                            in1=g44[:, cs_, :, 0], op=OP.mult)
                    nc.vector.tensor_reduce(out=n0t[:, t0 + tch:t0 + te],
                                            in_=m3a[:, cs_], axis=AX.X, op=OP.add)
                    nc.vector.tensor_tensor(out=m3a[:, cs_], in0=ex3a[:, cs_],
                                            in1=g44[:, cs_, :, 1], op=OP.mult)
                    nc.vector.tensor_reduce(out=n1t[:, t0 + tch:t0 + te],
                                            in_=m3a[:, cs_], axis=AX.X, op=OP.add)

            # ---- out2 = [n0,n1]/(d2+1e-16); masked logsoftmax partial sums
            nc.vector.tensor_scalar_add(out=d2t[:], in0=d2t[:], scalar1=1e-16)
            r2 = pool.tile([P, T], F32)
            nc.vector.reciprocal(out=r2[:], in_=d2t[:])
            o0 = pool.tile([P, T], F32)
            o1 = pool.tile([P, T], F32)
            nc.vector.tensor_tensor(out=o0[:], in0=n0t[:], in1=r2[:], op=OP.mult)
            nc.vector.tensor_tensor(out=o1[:], in0=n1t[:], in1=r2[:], op=OP.mult)
            e0 = pool.tile([P, T], F32)
            e1 = pool.tile([P, T], F32)
            nc.scalar.activation(e0[:], o0[:], AF.Exp)
            nc.scalar.activation(e1[:], o1[:], AF.Exp)
            nc.vector.tensor_tensor(out=e0[:], in0=e0[:], in1=e1[:], op=OP.add)
            lse = pool.tile([P, T], F32)
            nc.scalar.activation(lse[:], e0[:], AF.Ln)
            part = pool.tile([P, 4], F32)
            for cc, src_t in enumerate((o0, o1, lse)):
                nc.vector.tensor_tensor(out=src_t[:], in0=src_t[:],
                                        in1=sb["nmT"][:], op=OP.mult)
                nc.vector.tensor_reduce(out=part[:, cc:cc + 1], in_=src_t[:],
                                        axis=AX.X, op=OP.add)
            nc.vector.memset(part[:, 3:4], 0)
            nc.sync.dma_start(out=out_part[:], in_=part[:])

    nc.compile()
    return nc


def _finish(parts):
    r = np.stack([np.asarray(p, np.float64) for p in parts]).sum(axis=(0, 1))
    return np.array([[(r[0] - r[2]) / N, (r[1] - r[2]) / N]], np.float32)


# ---------------------------------------------------------------- fallback
def _kernel_numpy(x, edge_index, W1, a_src1, a_dst1, b1, W2, a_src2, a_dst2, b2):
    SLOPE = 0.2

    def lrelu(v):
        return np.where(v >= 0, v, SLOPE * v)

    def gat_conv(h, W, a_src, a_dst, b, src_s, dst_s, starts, heads, out_ch):
        n = h.shape[0]
        hp = (h @ W).reshape(n, heads, out_ch)
        al_s = (hp * a_src[None]).sum(-1)
        al_d = (hp * a_dst[None]).sum(-1)
        e = lrelu(al_s[src_s] + al_d[dst_s])
        emax = np.maximum.reduceat(e, starts, axis=0)
        ex = np.exp(e - emax[dst_s])
        denom = np.add.reduceat(ex, starts, axis=0)
        alpha = ex / (denom[dst_s] + 1e-16)
        out = np.empty((n, heads * out_ch), np.float32)
        BLK = 8192
        Et = src_s.shape[0]
        st = np.asarray(starts)
        for nb in range(0, n, BLK):
            ne = min(nb + BLK, n)
            r0 = st[nb]
            r1 = st[ne] if ne < n else Et
            w = (alpha[r0:r1, :, None] * hp[src_s[r0:r1]]).reshape(r1 - r0, -1)
            out[nb:ne] = np.add.reduceat(w, st[nb:ne] - r0, axis=0)
        return out + b

    x = np.asarray(x, np.float32)
    ei = np.asarray(edge_index)
    n = x.shape[0]
    loop = np.arange(n, dtype=np.int64)
    src = np.concatenate([ei[0].astype(np.int64), loop])
    dst = np.concatenate([ei[1].astype(np.int64), loop])
    order = np.argsort(dst, kind="stable")
    src_s, dst_s = src[order], dst[order]
    starts = np.searchsorted(dst_s, np.arange(n, dtype=np.int64))
    h1 = gat_conv(x, np.asarray(W1, np.float32), np.asarray(a_src1, np.float32),
                  np.asarray(a_dst1, np.float32), np.asarray(b1, np.float32),
                  src_s, dst_s, starts, 8, 64)
    h1 = np.maximum(h1, 0.0)
    h2 = gat_conv(h1, np.asarray(W2, np.float32), np.asarray(a_src2, np.float32),
                  np.asarray(a_dst2, np.float32), np.asarray(b2, np.float32),
                  src_s, dst_s, starts, 1, 2)
    m = h2.max(axis=1, keepdims=True)
    z = h2 - m
    ls = z - np.log(np.exp(z).sum(axis=1, keepdims=True))
    return ls.mean(axis=0, dtype=np.float64).astype(np.float32)[None, :]


# ---------------------------------------------------------------- entry
_CACHE = {}


def kernel(x, edge_index, W1, a_src1, a_dst1, b1, W2, a_src2, a_dst2, b2):
    global LAST_EXEC_NS
    try:
        assert np.asarray(x).shape == (N, 1)
        assert np.asarray(edge_index).shape == (2, E)
        assert np.all(np.asarray(b1) == 0) and np.all(np.asarray(b2) == 0)

        from concourse.bass_utils import run_bass_kernel_spmd

        meta, cores = _host_prep(x, edge_index, W1, a_src1, a_dst1,
                                 W2, a_src2, a_dst2)
        key = (meta["K1"], meta["K2"], meta["T1"], meta["T2"])
        if key not in _CACHE:
            _CACHE[key] = _build_program(meta)
        nc = _CACHE[key]

        in_maps = [dict(c) for c in cores]
        trace = bool(int(os.environ.get("GAT_TRACE", "0")))
        kw = {}
        if trace:
            kw["trace"] = True
            kw["trace_cores"] = list(range(NC))
        res = run_bass_kernel_spmd(nc, in_maps, list(range(NC)), **kw)
        LAST_EXEC_NS = res.exec_time_ns
        parts = [res.results[i]["part"] for i in range(NC)]
        out = _finish(parts)
        if not np.all(np.isfinite(out)):
            raise RuntimeError("non-finite device output")
        return out
    except Exception:
        import traceback
        traceback.print_exc()
        return _kernel_numpy(x, edge_index, W1, a_src1, a_dst1, b1,
                             W2, a_src2, a_dst2, b2)



# revision 2
# speedup vs baseline: 38329.9288x; 38329.9288x over previous
"""2-layer GAT (nn_GAT_88381837017178) on 8 trn2 NeuronCores via Bass/Tile.

Takes FULL unsharded inputs, returns the FULL [1,2] output.

Math: with x [N,1], h1 = x @ W1 is rank-1, so each head's attention
logits are affine in x and layer-1's aggregated output per head is
s1[n,h] * W1row[h,:], where s1 is the attention-weighted sum of x[src].
relu splits rank-2: relu(s1*W1row) = pos(s1)*posW1 + neg(s1)*negW1, so
layer 2 only needs a per-node 4-vector [h2_0, h2_1, as2, ad2], linear in
[pos(s1), neg(s1)] (valid because b1 == b2 == 0; asserted).

Sharding (edge parallelism per the dst-sorted contiguous split): nodes
are split over 8 cores; each core owns a dst-complete edge shard laid
out as dense node-padded [128, T, K] tiles (two degree buckets).  The
segment softmax becomes dense DVE/ACT tile ops.  Layer 2 needs node
features of arbitrary global src nodes: per-core [NPPC,4] table slices
are AllGather'd across the 8 cores, then an indirect (gather) DMA pulls
per-edge rows, and a second dense softmax + logsoftmax partial-sum
epilogue finishes on-device.  Host only sorts/pads indices, gathers the
raw input x per edge slot, and sums the 8 [128,4] per-core partials.

If anything in the device path fails (different shapes, nonzero biases,
wedged device), kernel() falls back to an equivalent numpy
implementation so correctness is preserved.
"""
import os
import numpy as np

N = 50000
E = 400000
NC = 8
NPC = N // NC
H1, F1 = 8, 64
P = 128

LAST_EXEC_NS = None


def _ceil(a, b):
    return -(-a // b)


# ---------------------------------------------------------------- host prep
def _host_prep(x, edge_index, W1, a_src1, a_dst1, W2, a_src2, a_dst2):
    import ml_dtypes

    x = np.asarray(x, np.float32).reshape(-1)
    ei = np.asarray(edge_index)
    W1 = np.asarray(W1, np.float32).reshape(H1, F1)
    a_src1 = np.asarray(a_src1, np.float32)
    a_dst1 = np.asarray(a_dst1, np.float32)
    W2 = np.asarray(W2, np.float32).reshape(H1, F1, 2)
    a_src2 = np.asarray(a_src2, np.float32).reshape(2)
    a_dst2 = np.asarray(a_dst2, np.float32).reshape(2)

    loop = np.arange(N, dtype=np.int64)
    src = np.concatenate([ei[0].astype(np.int64), loop])
    dst = np.concatenate([ei[1].astype(np.int64), loop])
    order = np.argsort(dst, kind="stable")
    src_s = src[order]
    deg = np.bincount(dst, minlength=N).astype(np.int64)
    starts = np.zeros(N + 1, np.int64)
    np.cumsum(deg, out=starts[1:])

    kmax = int(deg.max())
    K2 = _ceil(kmax, 4) * 4
    best = None
    for K1 in range(8, min(K2, 28) + 1, 4):
        degc = deg.reshape(NC, NPC)
        c1 = (degc <= K1).sum(axis=1)
        c2 = NPC - c1
        T1 = _ceil(int(c1.max()), P)
        T2 = _ceil(int(c2.max()), P) + 1     # +1 pad tile guarantees dummy row
        slots = T1 * P * K1 + T2 * P * K2
        if best is None or slots < best[0]:
            best = (slots, K1, T1, T2)
    _, K1, T1, T2 = best
    T = T1 + T2
    NPPC = T * P
    DUMMY = NPPC - 1

    cs = np.einsum("hf,hf->h", W1, a_src1).astype(np.float32)
    cd = np.einsum("hf,hf->h", W1, a_dst1).astype(np.float32)
    A = np.einsum("hf,hfc->hc", np.maximum(W1, 0.0), W2)
    Bm = np.einsum("hf,hfc->hc", np.maximum(-W1, 0.0), W2)
    wvec = np.zeros((4, 16), np.float32)
    wvec[0] = np.concatenate([A[:, 0], Bm[:, 0]])
    wvec[1] = np.concatenate([A[:, 1], Bm[:, 1]])
    wvec[2] = np.concatenate([A @ a_src2, Bm @ a_src2])
    wvec[3] = np.concatenate([A @ a_dst2, Bm @ a_dst2])

    def to_pt(v):
        return np.ascontiguousarray(v.reshape(T, P).T)

    def to_ptk(v, Tb, Kb):
        return np.ascontiguousarray(
            v.reshape(Tb, P, Kb).transpose(1, 0, 2).reshape(P, Tb * Kb))

    tabrow = np.zeros(N, np.int64)
    percore = []
    for c in range(NC):
        n0 = c * NPC
        degc = deg[n0:n0 + NPC]
        b1_ids = np.nonzero(degc <= K1)[0] + n0
        b2_ids = np.nonzero(degc > K1)[0] + n0
        rows = np.zeros(NPC, np.int64)
        rows[b1_ids - n0] = np.arange(len(b1_ids))
        rows[b2_ids - n0] = T1 * P + np.arange(len(b2_ids))
        tabrow[n0:n0 + NPC] = c * NPPC + rows
        percore.append((b1_ids, b2_ids, rows))

    cores = []
    for c in range(NC):
        n0 = c * NPC
        b1_ids, b2_ids, rows = percore[c]
        xd = np.zeros(NPPC, np.float32)
        nm = np.zeros(NPPC, np.float32)
        pc_ = np.zeros(NPPC, np.float32)
        xd[rows] = x[n0:n0 + NPC]
        nm[rows] = 1.0
        kb_of_row = np.where(np.arange(NPPC) < T1 * P, K1, K2).astype(np.float32)
        pc_[:] = kb_of_row - 1.0             # pad rows pretend degree 1
        pc_[rows] = kb_of_row[rows] - deg[n0:n0 + NPC]

        def edge_tables(ids, Tb, Kb, row_off):
            xs = np.zeros((Tb * P, Kb), np.float32)
            gi = np.full((Tb * P, Kb), DUMMY, np.int64)
            if len(ids):
                d = deg[ids]
                rep_rows = np.repeat(rows[ids - n0] - row_off, d)
                k = np.arange(d.sum()) - np.repeat(
                    np.concatenate([[0], d.cumsum()[:-1]]), d)
                epos = np.repeat(starts[ids], d) + k
                s_nodes = src_s[epos]
                xs[rep_rows, k] = x[s_nodes]
                gi[rep_rows, k] = tabrow[s_nodes]
            return to_ptk(xs, Tb, Kb), to_ptk(gi, Tb, Kb).astype(np.int32)

        xs1, gi1 = edge_tables(b1_ids, T1, K1, 0)
        xs2, gi2 = edge_tables(b2_ids, T2, K2, T1 * P)
        cores.append(dict(
            xs1=xs1.astype(ml_dtypes.bfloat16), gi1=gi1,
            xs2=xs2.astype(ml_dtypes.bfloat16), gi2=gi2,
            xdT=to_pt(xd), nmT=to_pt(nm), pcT=to_pt(pc_),
            csrep=np.tile(cs, (P, 1)), cdrep=np.tile(cd, (P, 1)),
            bsdrep=np.zeros((P, 8), np.float32),
            wvec=np.tile(wvec.reshape(1, 64), (P, 1)),
            dumrow=np.array([[0.0, 0.0, -1e30, 0.0]], ml_dtypes.bfloat16),
        ))

    meta = dict(K1=K1, K2=K2, T1=T1, T2=T2, T=T, NPPC=NPPC, DUMMY=DUMMY)
    return meta, cores


# ---------------------------------------------------------------- program
def _build_program(meta, gather_chunks=2):
    import concourse.bacc as bacc
    import concourse.tile as tile
    from concourse import mybir
    from concourse.bass import IndirectOffsetOnAxis

    F32 = mybir.dt.float32
    BF16 = mybir.dt.bfloat16
    I32 = mybir.dt.int32
    AX = mybir.AxisListType
    OP = mybir.AluOpType
    AF = mybir.ActivationFunctionType

    K1, K2, T1, T2, T = meta["K1"], meta["K2"], meta["T1"], meta["T2"], meta["T"]
    NPPC = meta["NPPC"]
    NTAB = NC * NPPC

    nc = bacc.Bacc("TRN2", target_bir_lowering=False, debug=False,
                   num_devices=NC)
    ins = {}
    for name, shape, dt in [
        ("xs1", [P, T1 * K1], BF16), ("gi1", [P, T1 * K1], I32),
        ("xs2", [P, T2 * K2], BF16), ("gi2", [P, T2 * K2], I32),
        ("xdT", [P, T], F32), ("nmT", [P, T], F32), ("pcT", [P, T], F32),
        ("csrep", [P, 8], F32), ("cdrep", [P, 8], F32), ("bsdrep", [P, 8], F32),
        ("wvec", [P, 64], F32), ("dumrow", [1, 4], BF16),
    ]:
        ins[name] = nc.dram_tensor(name, shape, dt, kind="ExternalInput").ap()
    out_part = nc.dram_tensor("part", [P, 4], F32, kind="ExternalOutput").ap()

    _lp = nc.allow_low_precision("bf16 edge softmax; final tolerance 2e-2")
    _lp.__enter__()
    with tile.TileContext(nc) as tc:
        with tc.tile_pool(name="sb", bufs=1) as pool, \
             tc.tile_pool(name="dram", bufs=1, space="DRAM") as dram:
            sb = {}
            order = ["dumrow", "xdT", "csrep", "cdrep", "bsdrep",
                     "pcT", "nmT", "wvec", "xs1", "xs2", "gi1", "gi2"]
            wu = {}
            for name in order:
                ap = ins[name]
                t_ = pool.tile(list(ap.shape), ap.dtype, name=f"sb_{name}")
                nc.sync.dma_start(out=t_[:], in_=ap[:])
                sb[name] = t_
                if name == "dumrow":
                    # warm-up collective first: absorbs core launch skew and
                    # ncfw first-use cost while the big input DMAs stream in
                    wu["in"] = dram.tile([1, 4], BF16, name="wuin")
                    wu["out"] = dram.tile([NC, 4], BF16, name="wuout",
                                          addr_space="Shared")
                    nc.sync.dma_start(out=wu["in"][:], in_=t_[:])
                    nc.gpsimd.collective_compute(
                        "AllGather", OP.bypass,
                        replica_groups=[list(range(NC))],
                        ins=[wu["in"].opt()], outs=[wu["out"].opt()])

            buckets = [
                (sb["xs1"], sb["gi1"], T1, K1, 0),
                (sb["xs2"], sb["gi2"], T2, K2, T1),
            ]

            xdcd = pool.tile([P, 8 * T], F32)
            xdcd3 = xdcd[:].rearrange("p (h t) -> p h t", h=8)
            for h in range(8):
                nc.scalar.activation(xdcd3[:, h, :], sb["xdT"][:], AF.Identity,
                                     bias=sb["bsdrep"][:, h:h + 1],
                                     scale=sb["cdrep"][:, h:h + 1])

            num = pool.tile([P, T * 8], BF16)
            den = pool.tile([P, T * 8], BF16)
            num3 = num[:].rearrange("p (h t) -> p h t", h=8)
            den3 = den[:].rearrange("p (h t) -> p h t", h=8)

            scratch = {}
            for bi, (xs_sb, gi_sb, Tb, Kb, t0) in enumerate(buckets):
                scratch[bi] = dict(
                    u=pool.tile([P, Tb * Kb], BF16, name=f"u_{bi}"),
                    l=pool.tile([P, Tb * Kb], BF16, name=f"l_{bi}"),
                    ex=pool.tile([P, Tb * Kb], BF16, name=f"ex_{bi}"),
                    m=pool.tile([P, Tb * Kb], BF16, name=f"m_{bi}"),
                )

            # ---- layer-1 edge phase (dense segment softmax, all 8 heads)
            # per-head u tiles so all Lrelu ops batch under one ACT table
            ut = {}
            for h in range(8):
                for bi, (xs_sb, gi_sb, Tb, Kb, t0) in enumerate(buckets):
                    ut[h, bi] = pool.tile([P, Tb * Kb], BF16, name=f"u{h}_{bi}")
            for h in range(8):
                for bi, (xs_sb, gi_sb, Tb, Kb, t0) in enumerate(buckets):
                    xs3 = xs_sb[:].rearrange("p (t k) -> p t k", k=Kb)
                    u3 = ut[h, bi][:].rearrange("p (t k) -> p t k", k=Kb)
                    xdcd_b = xdcd3[:, h, t0:t0 + Tb].to_broadcast([P, Tb, Kb])
                    nc.vector.scalar_tensor_tensor(
                        out=u3, in0=xs3, scalar=sb["csrep"][:, h:h + 1],
                        in1=xdcd_b, op0=OP.mult, op1=OP.add)
            # all leaky-relus back-to-back (single ACT table load)
            for h in range(8):
                for bi in range(len(buckets)):
                    nc.scalar.activation(ut[h, bi][:], ut[h, bi][:],
                                         AF.Lrelu, alpha=0.2)
                nc.scalar.activation(xdcd3[:, h, :], xdcd3[:, h, :],
                                     AF.Lrelu, alpha=0.2)
            # exp + weighted sums (back on the exp table)
            for h in range(8):
                for bi, (xs_sb, gi_sb, Tb, Kb, t0) in enumerate(buckets):
                    ss = scratch[bi]
                    nc.scalar.activation(ss["ex"][:], ut[h, bi][:], AF.Exp)
                    nc.vector.tensor_tensor(out=ss["m"][:], in0=ss["ex"][:],
                                            in1=xs_sb[:], op=OP.mult)
                    m3 = ss["m"][:].rearrange("p (t k) -> p t k", k=Kb)
                    ex3 = ss["ex"][:].rearrange("p (t k) -> p t k", k=Kb)
                    nc.vector.tensor_reduce(out=num3[:, h, t0:t0 + Tb],
                                            in_=m3, axis=AX.X, op=OP.add)
                    nc.vector.tensor_reduce(out=den3[:, h, t0:t0 + Tb],
                                            in_=ex3, axis=AX.X, op=OP.add)
                # pad-slot correction (xdcd already leaky-relu'd in place)
                et = pool.tile([P, T], BF16, name="et", tag="et")
                nc.scalar.activation(et[:], xdcd3[:, h, :], AF.Exp)
                nc.vector.tensor_tensor(out=et[:], in0=et[:],
                                        in1=sb["pcT"][:], op=OP.mult)
                nc.vector.tensor_tensor(out=den3[:, h, :], in0=den3[:, h, :],
                                        in1=et[:], op=OP.subtract)

            # ---- s1 = num/den; pm = [relu(s1), relu(-s1)]
            rec = pool.tile([P, T * 8], F32)
            nc.vector.reciprocal(out=rec[:], in_=den[:])
            s1 = pool.tile([P, T * 8], F32)
            nc.vector.tensor_tensor(out=s1[:], in0=num[:], in1=rec[:], op=OP.mult)
            s13 = s1[:].rearrange("p (h t) -> p t h", h=8)
            pm = pool.tile([P, T * 16], F32)
            pm3 = pm[:].rearrange("p (t j) -> p t j", j=16)
            nc.scalar.activation(pm3[:, :, 0:8], s13, AF.Relu)
            nc.scalar.activation(pm3[:, :, 8:16], s13, AF.Relu, scale=-1.0)

            # ---- per-node table row [h2_0, h2_1, as2, ad2] = pm @ wvec.T
            cols = pool.tile([P, T * 4], BF16)
            cols3 = cols[:].rearrange("p (t c) -> p t c", c=4)
            wv4 = sb["wvec"][:].rearrange("p (c j) -> p c j", c=4)
            for cc in range(4):
                pr = pool.tile([P, T * 16], F32, name="pr", tag="pr")
                pr3 = pr[:].rearrange("p (t j) -> p t j", j=16)
                wb = wv4[:, cc, :].unsqueeze(1).to_broadcast([P, T, 16])
                nc.vector.tensor_tensor(out=pr3, in0=pm3, in1=wb, op=OP.mult)
                nc.vector.tensor_reduce(out=cols3[:, :, cc], in_=pr3,
                                        axis=AX.X, op=OP.add)

            # ---- local table slice -> AllGather -> full table
            tabloc = dram.tile([NPPC, 4], BF16)
            tabv = tabloc[:].rearrange("(t p) c -> p t c", p=P)
            nc.sync.dma_start(out=tabv, in_=cols3)
            # every core's last row is padding; it is the gather dummy row
            nc.sync.dma_start(out=tabloc[NPPC - 1:NPPC, :], in_=sb["dumrow"][:])
            tabfull = dram.tile([NTAB, 4], BF16, addr_space="Shared")
            nc.gpsimd.collective_compute(
                "AllGather", OP.bypass, replica_groups=[list(range(NC))],
                ins=[tabloc.opt()], outs=[tabfull.opt()])

            # ---- layer-2: chunked indirect gather + dense segment softmax
            n0t = pool.tile([P, T], F32)
            n1t = pool.tile([P, T], F32)
            d2t = pool.tile([P, T], F32)
            for bi, (xs_sb, gi_sb, Tb, Kb, t0) in enumerate(buckets):
                g4 = pool.tile([P, Tb * Kb * 4], BF16, name=f"g4_{bi}")
                g44 = g4[:].rearrange("p (t k c) -> p t k c", k=Kb, c=4)
                s = scratch[bi]
                u3a = s["u"][:].rearrange("p (t k) -> p t k", k=Kb)
                l3a = s["l"][:].rearrange("p (t k) -> p t k", k=Kb)
                ex3a = s["ex"][:].rearrange("p (t k) -> p t k", k=Kb)
                m3a = s["m"][:].rearrange("p (t k) -> p t k", k=Kb)
                step = _ceil(Tb, gather_chunks)
                for tch in range(0, Tb, step):
                    te = min(tch + step, Tb)
                    cs_ = slice(tch, te)
                    nc.gpsimd.indirect_dma_start(
                        out=g4[:, tch * Kb * 4:te * Kb * 4],
                        out_offset=None,
                        in_=tabfull[:],
                        in_offset=IndirectOffsetOnAxis(
                            ap=gi_sb[:, tch * Kb:te * Kb], axis=0))
                    ad2b = cols3[:, t0 + tch:t0 + te, 3].to_broadcast(
                        [P, te - tch, Kb])
                    nc.vector.tensor_tensor(out=u3a[:, cs_], in0=g44[:, cs_, :, 2],
                                            in1=ad2b, op=OP.add)
                    nc.scalar.activation(l3a[:, cs_], u3a[:, cs_], AF.Exp, scale=0.2)
                    nc.scalar.activation(ex3a[:, cs_], u3a[:, cs_], AF.Exp)
                    nc.vector.tensor_tensor(out=ex3a[:, cs_], in0=ex3a[:, cs_],
                                            in1=l3a[:, cs_], op=OP.max)
                    nc.vector.tensor_reduce(out=d2t[:, t0 + tch:t0 + te],
                                            in_=ex3a[:, cs_], axis=AX.X, op=OP.add)
                    nc.vector.tensor_tensor(out=m3a[:, cs_], in0=ex3a[:, cs_],
                                            in1=g44[:, cs_, :, 0], op=OP.mult)
                    nc.vector.tensor_reduce(out=n0t[:, t0 + tch:t0 + te],
                                            in_=m3a[:, cs_], axis=AX.X, op=OP.add)
                    nc.vector.tensor_tensor(out=m3a[:, cs_], in0=ex3a[:, cs_],
                                            in1=g44[:, cs_, :, 1], op=OP.mult)
                    nc.vector.tensor_reduce(out=n1t[:, t0 + tch:t0 + te],
                                            in_=m3a[:, cs_], axis=AX.X, op=OP.add)

            # ---- out2 = [n0,n1]/(d2+1e-16); masked logsoftmax partial sums
            nc.vector.tensor_scalar_add(out=d2t[:], in0=d2t[:], scalar1=1e-16)
            r2 = pool.tile([P, T], F32)
            nc.vector.reciprocal(out=r2[:], in_=d2t[:])
            o0 = pool.tile([P, T], F32)
            o1 = pool.tile([P, T], F32)
            nc.vector.tensor_tensor(out=o0[:], in0=n0t[:], in1=r2[:], op=OP.mult)
            nc.vector.tensor_tensor(out=o1[:], in0=n1t[:], in1=r2[:], op=OP.mult)
            e0 = pool.tile([P, T], F32)
            e1 = pool.tile([P, T], F32)
            nc.scalar.activation(e0[:], o0[:], AF.Exp)
            nc.scalar.activation(e1[:], o1[:], AF.Exp)
            nc.vector.tensor_tensor(out=e0[:], in0=e0[:], in1=e1[:], op=OP.add)
            lse = pool.tile([P, T], F32)
            nc.scalar.activation(lse[:], e0[:], AF.Ln)
            part = pool.tile([P, 4], F32)
            for cc, src_t in enumerate((o0, o1, lse)):
                nc.vector.tensor_tensor(out=src_t[:], in0=src_t[:],
                                        in1=sb["nmT"][:], op=OP.mult)
                nc.vector.tensor_reduce(out=part[:, cc:cc + 1], in_=src_t[:],
                                        axis=AX.X, op=OP.add)
            nc.vector.memset(part[:, 3:4], 0)
            nc.sync.dma_start(out=out_part[:], in_=part[:])

    nc.compile()
    return nc


def _finish(parts):
    r = np.stack([np.asarray(p, np.float64) for p in parts]).sum(axis=(0, 1))
    return np.array([[(r[0] - r[2]) / N, (r[1] - r[2]) / N]], np.float32)


# ---------------------------------------------------------------- fallback
def _kernel_numpy(x, edge_index, W1, a_src1, a_dst1, b1, W2, a_src2, a_dst2, b2):
    SLOPE = 0.2

    def lrelu(v):
        return np.where(v >= 0, v, SLOPE * v)

    def gat_conv(h, W, a_src, a_dst, b, src_s, dst_s, starts, heads, out_ch):
        n = h.shape[0]
        hp = (h @ W).reshape(n, heads, out_ch)
        al_s = (hp * a_src[None]).sum(-1)
        al_d = (hp * a_dst[None]).sum(-1)
        e = lrelu(al_s[src_s] + al_d[dst_s])
        emax = np.maximum.reduceat(e, starts, axis=0)
        ex = np.exp(e - emax[dst_s])
        denom = np.add.reduceat(ex, starts, axis=0)
        alpha = ex / (denom[dst_s] + 1e-16)
        out = np.empty((n, heads * out_ch), np.float32)
        BLK = 8192
        Et = src_s.shape[0]
        st = np.asarray(starts)
        for nb in range(0, n, BLK):
            ne = min(nb + BLK, n)
            r0 = st[nb]
            r1 = st[ne] if ne < n else Et
            w = (alpha[r0:r1, :, None] * hp[src_s[r0:r1]]).reshape(r1 - r0, -1)
            out[nb:ne] = np.add.reduceat(w, st[nb:ne] - r0, axis=0)
        return out + b

    x = np.asarray(x, np.float32)
    ei = np.asarray(edge_index)
    n = x.shape[0]
    loop = np.arange(n, dtype=np.int64)
    src = np.concatenate([ei[0].astype(np.int64), loop])
    dst = np.concatenate([ei[1].astype(np.int64), loop])
    order = np.argsort(dst, kind="stable")
    src_s, dst_s = src[order], dst[order]
    starts = np.searchsorted(dst_s, np.arange(n, dtype=np.int64))
    h1 = gat_conv(x, np.asarray(W1, np.float32), np.asarray(a_src1, np.float32),
                  np.asarray(a_dst1, np.float32), np.asarray(b1, np.float32),
                  src_s, dst_s, starts, 8, 64)
    h1 = np.maximum(h1, 0.0)
    h2 = gat_conv(h1, np.asarray(W2, np.float32), np.asarray(a_src2, np.float32),
                  np.asarray(a_dst2, np.float32), np.asarray(b2, np.float32),
                  src_s, dst_s, starts, 1, 2)
    m = h2.max(axis=1, keepdims=True)
    z = h2 - m
    ls = z - np.log(np.exp(z).sum(axis=1, keepdims=True))
    return ls.mean(axis=0, dtype=np.float64).astype(np.float32)[None, :]


# ---------------------------------------------------------------- entry
_CACHE = {}


def kernel(x, edge_index, W1, a_src1, a_dst1, b1, W2, a_src2, a_dst2, b2):
    global LAST_EXEC_NS
    try:
        assert np.asarray(x).shape == (N, 1)
        assert np.asarray(edge_index).shape == (2, E)
        assert np.all(np.asarray(b1) == 0) and np.all(np.asarray(b2) == 0)

        from concourse.bass_utils import run_bass_kernel_spmd

        meta, cores = _host_prep(x, edge_index, W1, a_src1, a_dst1,
                                 W2, a_src2, a_dst2)
        key = (meta["K1"], meta["K2"], meta["T1"], meta["T2"])
        if key not in _CACHE:
            _CACHE[key] = _build_program(meta)
        nc = _CACHE[key]

        in_maps = [dict(c) for c in cores]
        trace = bool(int(os.environ.get("GAT_TRACE", "0")))
        kw = {}
        if trace:
            kw["trace"] = True
            kw["trace_cores"] = list(range(NC))
            td = os.environ.get("GAT_TRACE_DIR")
            if td:
                kw["tmpdir"] = td
        res = run_bass_kernel_spmd(nc, in_maps, list(range(NC)), **kw)
        LAST_EXEC_NS = res.exec_time_ns
        parts = [res.results[i]["part"] for i in range(NC)]
        out = _finish(parts)
        if not np.all(np.isfinite(out)):
            raise RuntimeError("non-finite device output")
        return out
    except Exception:
        import traceback
        traceback.print_exc()
        return _kernel_numpy(x, edge_index, W1, a_src1, a_dst1, b1,
                             W2, a_src2, a_dst2, b2)

